# revision 11
# baseline (speedup 1.0000x reference)
"""AttentiveItemToVec TRN2 kernel v5 (8 NeuronCores, SPMD data-parallel).

Host folds all linear layers / norms / masks into gather tables:
  ttab [V, 40]  bf16 = rows (tvec@At_w.T + At_b) / max(||.||, eps)
  ctab [V, 172] bf16 = [ cvec@W2.T (128) | 1.0 | pad(3) |
                         (cvec@Ac_w.T + Ac_b)/max(||.||,eps) (40) ]
  (W2 = R_w@Bc_w; b2 = R_w@Bc_b + R_b added at the end; cosine = dot of
   pre-normalized rows; a ones column makes the z matmul also emit the
   softmax row-sum.)

Device, all token-major (tokens = flattened (b, m), 128 per tile):
  - 100 c-gathers + 32 t-gathers (indirect DMA ~1.4us/instr cadence on
    gpsimd = the bottleneck; everything else hides underneath)
  - PE transposes ckn/tq into rows 0..39 of ckTn_ext [43, 12800] /
    tqnT_ext [43, 4096]; rows 40..42 are host-built rank-3 bias factors:
    the dot matmul contracts over 43 rows and lands cos + pad mask +
    cross-b kill (-1e30) in one op -> PSUM [128, 32*g] (g = 2..3 batch
    rows per tile)
  - one bias-free exp per tile -> et bf16; per-(tile, b) z matmul
    accumulating into 32-row band (b%3) of a shared [96, 129] PSUM tile
  - per 3 b's: reciprocal + scale + bias-add + one [96, 128] DMA out
  - software-pipelined with a 2-tile stage lag so no engine queue blocks
    on same-iteration producers.
"""
import sys

sys.path.insert(0, "/opt/trn_rl_repo")

import numpy as np
import ml_dtypes

import concourse.bass as bass
import concourse.mybir as mybir
from concourse import bacc
from concourse.tile import TileContext
from concourse.bass_utils import run_bass_kernel_spmd

F32 = mybir.dt.float32
BF16 = mybir.dt.bfloat16
I32 = mybir.dt.int32
AF = mybir.ActivationFunctionType
OP = mybir.AluOpType

V, E, DA = 1_000_000, 128, 40
B, J, M = 1024, 32, 100
NCORES = 8
BL = B // NCORES          # 128 batch rows per core
CW = 172                  # ctab row: [bu2 128 | one | pad 3 | ckn 40]
DX = DA + 3               # contraction rows incl. rank-3 bias factors
NT_C = BL * M // 128      # 100 c-gather tiles
NT_T = BL * J // 128      # 32 t-gather tiles
NEG = -1e30
EPS = 1e-6
NGRP = (BL + 2) // 3      # 43 output groups of <=3 batch rows

_trace = [False]
_last_exec_ns = [None]


def _bfirst(s):
    return (128 * s) // M


def _blast(s):
    return (128 * s + 127) // M


def _build_bass():
    nc = bacc.Bacc("TRN2", target_bir_lowering=False, debug=False,
                   num_devices=NCORES)

    ctab = nc.declare_dram_parameter("ctab", [V, CW], BF16, isOutput=False)
    ttab = nc.declare_dram_parameter("ttab", [V, DA], BF16, isOutput=False)
    cidx = nc.declare_dram_parameter("cidx", [128, NT_C], I32, isOutput=False)
    tidx = nc.declare_dram_parameter("tidx", [128, NT_T], I32, isOutput=False)
    negs3d = nc.declare_dram_parameter("negs3d", [3, BL * M], BF16,
                                       isOutput=False)
    bonesd = nc.declare_dram_parameter("bonesd", [3, BL * J], BF16,
                                       isOutput=False)
    b2d = nc.declare_dram_parameter("b2d", [96, E], F32, isOutput=False)
    identd = nc.declare_dram_parameter("identd", [128, 128], BF16,
                                       isOutput=False)
    zout = nc.declare_dram_parameter("zout", [BL, J, E], F32, isOutput=True)

    with TileContext(nc) as tc:
        from contextlib import ExitStack
        ctx = ExitStack()
        cp = ctx.enter_context(tc.tile_pool(name="const", bufs=1))
        bigp = ctx.enter_context(tc.tile_pool(name="big", bufs=1))
        crawp = ctx.enter_context(tc.tile_pool(name="craw", bufs=12))
        trawp = ctx.enter_context(tc.tile_pool(name="traw", bufs=6))
        etp = ctx.enter_context(tc.tile_pool(name="et", bufs=6))
        workp = ctx.enter_context(tc.tile_pool(name="work", bufs=4))
        tpps = ctx.enter_context(tc.tile_pool(name="tpps", bufs=2, space="PSUM"))
        dotps = ctx.enter_context(tc.tile_pool(name="dotps", bufs=3, space="PSUM"))
        zps_p = ctx.enter_context(tc.tile_pool(name="zps", bufs=3, space="PSUM"))

        # ---------------- constants (cidx first: unblocks gather 0) -----
        cidx_t = cp.tile([128, NT_C], I32)
        nc.sync.dma_start(out=cidx_t[:], in_=cidx[:, :])
        tidx_t = cp.tile([128, NT_T], I32)
        nc.sync.dma_start(out=tidx_t[:], in_=tidx[:, :])
        ident = cp.tile([128, 128], BF16)
        nc.sync.dma_start(out=ident[:], in_=identd[:, :])
        b2_t = cp.tile([96, E], F32)
        nc.sync.dma_start(out=b2_t[:], in_=b2d[:, :])

        ckTn_ext = bigp.tile([DX, BL * M], BF16)    # 25.6KB/part
        tqnT_ext = bigp.tile([DX, BL * J], BF16)    # 8KB/part
        nc.sync.dma_start(out=ckTn_ext[DA:DX, :], in_=negs3d[:, :])
        nc.sync.dma_start(out=tqnT_ext[DA:DX, :], in_=bonesd[:, :])

        craw_tiles = {}
        zp3_tiles = {}
        et_tiles = {}

        def emit_t(k):
            t_raw = trawp.tile([128, DA], BF16, tag="traw", bufs=6)
            nc.gpsimd.indirect_dma_start(
                out=t_raw[:], out_offset=None, in_=ttab[:, :],
                in_offset=bass.IndirectOffsetOnAxis(
                    ap=tidx_t[:, k:k + 1], axis=0))
            tp = tpps.tile([DA, 128], BF16, space="PSUM", tag="tp", bufs=2)
            nc.tensor.transpose(tp[:], t_raw[:], ident[:])
            if k % 2 == 0:
                nc.scalar.copy(tqnT_ext[0:DA, k * 128:(k + 1) * 128], tp[:])
            else:
                nc.vector.tensor_copy(tqnT_ext[0:DA, k * 128:(k + 1) * 128],
                                      tp[:])

        def emit_c(s):
            c_raw = crawp.tile([128, CW], BF16, tag="craw", bufs=12)
            craw_tiles[s] = c_raw
            nc.gpsimd.indirect_dma_start(
                out=c_raw[:], out_offset=None, in_=ctab[:, :],
                in_offset=bass.IndirectOffsetOnAxis(
                    ap=cidx_t[:, s:s + 1], axis=0))
            kp = tpps.tile([DA, 128], BF16, space="PSUM", tag="tp", bufs=2)
            nc.tensor.transpose(kp[:], c_raw[:, 132:CW], ident[:])
            if s % 2 == 0:
                nc.vector.tensor_copy(ckTn_ext[0:DA, s * 128:(s + 1) * 128],
                                      kp[:])
            else:
                nc.scalar.copy(ckTn_ext[0:DA, s * 128:(s + 1) * 128], kp[:])

        def emit_dot(s):
            b0, b1 = _bfirst(s), _blast(s)
            gcnt = b1 - b0 + 1
            dps = dotps.tile([128, 32 * gcnt], F32, space="PSUM",
                             tag="dot", bufs=3)
            nc.tensor.matmul(dps[:], ckTn_ext[:, s * 128:(s + 1) * 128],
                             tqnT_ext[:, b0 * J:(b1 + 1) * J],
                             start=True, stop=True)
            et = etp.tile([128, 32 * gcnt], BF16, tag="et", bufs=6)
            et_tiles[s] = et
            nc.scalar.activation(et[:], dps[:], AF.Exp, scale=1.0)

        def emit_z(s):
            b0, b1 = _bfirst(s), _blast(s)
            et = et_tiles.pop(s)
            for g in range(b1 - b0 + 1):
                b = b0 + g
                q = b % 3
                grp = b // 3
                if grp not in zp3_tiles:
                    zp3_tiles[grp] = zps_p.tile([96, E + 1], F32,
                                                space="PSUM", tag="z", bufs=3,
                                                name=f"zp3_{grp}")
                zp3 = zp3_tiles[grp]
                s0 = (b * M) // 128
                s1 = (b * M + M - 1) // 128
                nc.tensor.matmul(zp3[32 * q:32 * (q + 1), :],
                                 et[:, 32 * g:32 * (g + 1)],
                                 craw_tiles[s][:, 0:E + 1],
                                 start=(s == s0), stop=(s == s1))

        def emit_fin(grp):
            zp3 = zp3_tiles.pop(grp)
            nb = min(3 * grp + 3, BL) - 3 * grp     # 3, or 2 in last group
            r = 32 * nb
            inv = workp.tile([96, 1], F32, tag="inv", bufs=4)
            nc.vector.reciprocal(inv[:r], zp3[:r, E:E + 1])
            zsb = workp.tile([96, E], F32, tag="zsb", bufs=4)
            nc.vector.tensor_scalar_mul(zsb[:r], zp3[:r, 0:E], inv[:r, :1])
            nc.vector.tensor_tensor(out=zsb[:r], in0=zsb[:r],
                                    in1=b2_t[:r], op=OP.add)
            nc.sync.dma_start(out=zout[3 * grp:3 * grp + nb], in_=zsb[:r])

        # -------- software-pipelined schedule (2-tile stage lag) --------
        next_t = 0
        next_fin = 0
        for i in range(NT_C + 2):
            if i < NT_C:
                emit_c(i)
                if i % 3 == 1 and next_t < NT_T:
                    emit_t(next_t)
                    next_t += 1
                    if i == 1:          # front-load a second t tile
                        emit_t(next_t)
                        next_t += 1
            if 1 <= i <= NT_C:
                emit_dot(i - 1)
            if i >= 2:
                emit_z(i - 2)
                while next_fin < NGRP and \
                        (min(3 * next_fin + 2, BL - 1) * M + M - 1) // 128 \
                        <= i - 2:
                    emit_fin(next_fin)
                    next_fin += 1
        while next_t < NT_T:
            emit_t(next_t)
            next_t += 1
        while next_fin < NGRP:
            emit_fin(next_fin)
            next_fin += 1

        ctx.close()

    nc.finalize()
    return nc


_nc_cache = [None]


def kernel(batch_titems, batch_citems, pad_rows, pad_cols, tvec, cvec,
           Ac_w, Ac_b, At_w, At_b, Bc_w, Bc_b, R_w, R_b):
    batch_titems = np.asarray(batch_titems).astype(np.int32)
    batch_citems = np.asarray(batch_citems).astype(np.int32)
    pad_rows = np.asarray(pad_rows).astype(np.int64)
    pad_cols = np.asarray(pad_cols).astype(np.int64)
    tvec = np.asarray(tvec, dtype=np.float32)
    cvec = np.asarray(cvec, dtype=np.float32)
    Ac_w = np.asarray(Ac_w, dtype=np.float32)
    Ac_b = np.asarray(Ac_b, dtype=np.float32)
    At_w = np.asarray(At_w, dtype=np.float32)
    At_b = np.asarray(At_b, dtype=np.float32)
    Bc_w = np.asarray(Bc_w, dtype=np.float32)
    Bc_b = np.asarray(Bc_b, dtype=np.float32)
    R_w = np.asarray(R_w, dtype=np.float32)
    R_b = np.asarray(R_b, dtype=np.float32)

    # ---- host table folding ----
    W2 = R_w @ Bc_w                                   # [E, E]
    b2 = (R_w @ Bc_b + R_b).astype(np.float32)        # [E]
    bu2 = (cvec @ W2.T).astype(np.float32)            # [V, E]
    ck = cvec @ Ac_w.T + Ac_b                         # [V, DA]
    ck /= np.maximum(np.linalg.norm(ck, axis=1, keepdims=True), EPS)
    tq = tvec @ At_w.T + At_b                         # [V, DA]
    tq /= np.maximum(np.linalg.norm(tq, axis=1, keepdims=True), EPS)
    ttab = tq.astype(ml_dtypes.bfloat16)

    ctab = np.zeros((V, CW), dtype=ml_dtypes.bfloat16)
    ctab[:, 0:E] = bu2.astype(ml_dtypes.bfloat16)
    ctab[:, E] = np.asarray(1.0, dtype=ml_dtypes.bfloat16)
    ctab[:, 132:CW] = ck.astype(ml_dtypes.bfloat16)

    b2rep = np.broadcast_to(b2, (96, E)).copy()
    ident_np = np.eye(128, dtype=np.float32).astype(ml_dtypes.bfloat16)

    # block-ones rank-3 factor: [r, (b, j)] = 1 iff b % 3 == r
    bones = np.zeros((3, BL * J), dtype=np.float32)
    bb = np.repeat(np.arange(BL), J)
    bones[bb % 3, np.arange(BL * J)] = 1.0
    bones = bones.astype(ml_dtypes.bfloat16)

    in_maps = []
    toks = np.arange(BL * M)
    tok_b_own = toks // M
    for c in range(NCORES):
        b0c = c * BL
        cit = batch_citems[b0c:b0c + BL].ravel()      # [12800]
        tit = batch_titems[b0c:b0c + BL].ravel()      # [4096]
        cidx = np.ascontiguousarray(cit.reshape(NT_C, 128).T.astype(np.int32))
        tidx = np.ascontiguousarray(tit.reshape(NT_T, 128).T.astype(np.int32))
        sel = (pad_rows >= b0c) & (pad_rows < b0c + BL)
        negm = np.zeros((M, BL), dtype=np.float32)
        negm[pad_cols[sel], pad_rows[sel] - b0c] = NEG
        # rank-3 bias factor rows: negs3[r, tok] = bias of token tok w.r.t.
        # the unique b in its tile's group with b % 3 == r (NEG otherwise)
        negs3 = np.full((3, BL * M), NEG, dtype=np.float32)
        for s in range(NT_C):
            tsl = slice(128 * s, 128 * (s + 1))
            t = toks[tsl]
            for bg in range(_bfirst(s), _blast(s) + 1):
                own = tok_b_own[tsl] == bg
                col = np.full(128, NEG, dtype=np.float32)
                col[own] = negm[t[own] - M * bg, bg]
                negs3[bg % 3, tsl] = col
        in_maps.append({
            "ctab": ctab, "ttab": ttab,
            "cidx": cidx, "tidx": tidx,
            "negs3d": negs3.astype(ml_dtypes.bfloat16), "bonesd": bones,
            "b2d": b2rep, "identd": ident_np,
        })

    if _nc_cache[0] is None:
        _nc_cache[0] = _build_bass()
    nc = _nc_cache[0]

    res = run_bass_kernel_spmd(nc, in_maps, list(range(NCORES)),
                               trace=_trace[0])
    _last_exec_ns[0] = res.exec_time_ns
    z = np.concatenate([r["zout"] for r in res.results], axis=0)
    return z.astype(np.float32)


# revision 12
# speedup vs baseline: 1.0018x; 1.0018x over previous
"""AttentiveItemToVec TRN2 kernel v5 (8 NeuronCores, SPMD data-parallel).

Host folds all linear layers / norms / masks into gather tables:
  ttab [V, 40]  bf16 = rows (tvec@At_w.T + At_b) / max(||.||, eps)
  ctab [V, 172] bf16 = [ cvec@W2.T (128) | 1.0 | pad(3) |
                         (cvec@Ac_w.T + Ac_b)/max(||.||,eps) (40) ]
  (W2 = R_w@Bc_w; b2 = R_w@Bc_b + R_b added at the end; cosine = dot of
   pre-normalized rows; a ones column makes the z matmul also emit the
   softmax row-sum.)

Device, all token-major (tokens = flattened (b, m), 128 per tile):
  - 100 c-gathers + 32 t-gathers (indirect DMA ~1.4us/instr cadence on
    gpsimd = the bottleneck; everything else hides underneath)
  - PE transposes ckn/tq into rows 0..39 of ckTn_ext [43, 12800] /
    tqnT_ext [43, 4096]; rows 40..42 are host-built rank-3 bias factors:
    the dot matmul contracts over 43 rows and lands cos + pad mask +
    cross-b kill (-1e30) in one op -> PSUM [128, 32*g] (g = 2..3 batch
    rows per tile)
  - one bias-free exp per tile -> et bf16; per-(tile, b) z matmul
    accumulating into 32-row band (b%3) of a shared [96, 129] PSUM tile
  - per 3 b's: reciprocal + scale + bias-add + one [96, 128] DMA out
  - software-pipelined with a 2-tile stage lag so no engine queue blocks
    on same-iteration producers.
"""
import sys

sys.path.insert(0, "/opt/trn_rl_repo")

import numpy as np
import ml_dtypes

import concourse.bass as bass
import concourse.mybir as mybir
from concourse import bacc
from concourse.tile import TileContext
from concourse.bass_utils import run_bass_kernel_spmd

F32 = mybir.dt.float32
BF16 = mybir.dt.bfloat16
I32 = mybir.dt.int32
AF = mybir.ActivationFunctionType
OP = mybir.AluOpType

V, E, DA = 1_000_000, 128, 40
B, J, M = 1024, 32, 100
NCORES = 8
BL = B // NCORES          # 128 batch rows per core
CW = 172                  # ctab row: [bu2 128 | one | pad 3 | ckn 40]
DX = DA + 3               # contraction rows incl. rank-3 bias factors
NT_C = BL * M // 128      # 100 c-gather tiles
NT_T = BL * J // 128      # 32 t-gather tiles
NEG = -1e30
EPS = 1e-6
NGRP = (BL + 2) // 3      # 43 output groups of <=3 batch rows

_trace = [False]
_last_exec_ns = [None]


def _bfirst(s):
    return (128 * s) // M


def _blast(s):
    return (128 * s + 127) // M


def _build_bass():
    nc = bacc.Bacc("TRN2", target_bir_lowering=False, debug=False,
                   num_devices=NCORES)

    ctab = nc.declare_dram_parameter("ctab", [V, CW], BF16, isOutput=False)
    ttab = nc.declare_dram_parameter("ttab", [V, DA], BF16, isOutput=False)
    cidx = nc.declare_dram_parameter("cidx", [128, NT_C], I32, isOutput=False)
    tidx = nc.declare_dram_parameter("tidx", [128, NT_T], I32, isOutput=False)
    negs3d = nc.declare_dram_parameter("negs3d", [3, BL * M], BF16,
                                       isOutput=False)
    bonesd = nc.declare_dram_parameter("bonesd", [3, BL * J], BF16,
                                       isOutput=False)
    b2d = nc.declare_dram_parameter("b2d", [96, E], F32, isOutput=False)
    identd = nc.declare_dram_parameter("identd", [128, 128], BF16,
                                       isOutput=False)
    zout = nc.declare_dram_parameter("zout", [BL, J, E], F32, isOutput=True)

    with TileContext(nc) as tc:
        from contextlib import ExitStack
        ctx = ExitStack()
        cp = ctx.enter_context(tc.tile_pool(name="const", bufs=1))
        bigp = ctx.enter_context(tc.tile_pool(name="big", bufs=1))
        crawp = ctx.enter_context(tc.tile_pool(name="craw", bufs=12))
        trawp = ctx.enter_context(tc.tile_pool(name="traw", bufs=8))
        etp = ctx.enter_context(tc.tile_pool(name="et", bufs=6))
        workp = ctx.enter_context(tc.tile_pool(name="work", bufs=4))
        tpps = ctx.enter_context(tc.tile_pool(name="tpps", bufs=3, space="PSUM"))
        dotps = ctx.enter_context(tc.tile_pool(name="dotps", bufs=2, space="PSUM"))
        zps_p = ctx.enter_context(tc.tile_pool(name="zps", bufs=3, space="PSUM"))

        # ---------------- constants (cidx first: unblocks gather 0) -----
        cidx_t = cp.tile([128, NT_C], I32)
        nc.sync.dma_start(out=cidx_t[:], in_=cidx[:, :])
        tidx_t = cp.tile([128, NT_T], I32)
        nc.sync.dma_start(out=tidx_t[:], in_=tidx[:, :])
        ident = cp.tile([128, 128], BF16)
        nc.sync.dma_start(out=ident[:], in_=identd[:, :])
        b2_t = cp.tile([96, E], F32)
        nc.sync.dma_start(out=b2_t[:], in_=b2d[:, :])

        ckTn_ext = bigp.tile([DX, BL * M], BF16)    # 25.6KB/part
        tqnT_ext = bigp.tile([DX, BL * J], BF16)    # 8KB/part
        nc.sync.dma_start(out=ckTn_ext[DA:DX, :], in_=negs3d[:, :])
        nc.sync.dma_start(out=tqnT_ext[DA:DX, :], in_=bonesd[:, :])

        craw_tiles = {}
        zp3_tiles = {}
        et_tiles = {}

        def emit_t(k):
            t_raw = trawp.tile([128, DA], BF16, tag="traw", bufs=8)
            nc.gpsimd.indirect_dma_start(
                out=t_raw[:], out_offset=None, in_=ttab[:, :],
                in_offset=bass.IndirectOffsetOnAxis(
                    ap=tidx_t[:, k:k + 1], axis=0))
            tp = tpps.tile([DA, 128], BF16, space="PSUM", tag="tp", bufs=3)
            nc.tensor.transpose(tp[:], t_raw[:], ident[:])
            if k % 2 == 0:
                nc.scalar.copy(tqnT_ext[0:DA, k * 128:(k + 1) * 128], tp[:])
            else:
                nc.vector.tensor_copy(tqnT_ext[0:DA, k * 128:(k + 1) * 128],
                                      tp[:])

        def emit_c(s):
            c_raw = crawp.tile([128, CW], BF16, tag="craw", bufs=12)
            craw_tiles[s] = c_raw
            nc.gpsimd.indirect_dma_start(
                out=c_raw[:], out_offset=None, in_=ctab[:, :],
                in_offset=bass.IndirectOffsetOnAxis(
                    ap=cidx_t[:, s:s + 1], axis=0))
            kp = tpps.tile([DA, 128], BF16, space="PSUM", tag="tp", bufs=3)
            nc.tensor.transpose(kp[:], c_raw[:, 132:CW], ident[:])
            if s % 2 == 0:
                nc.vector.tensor_copy(ckTn_ext[0:DA, s * 128:(s + 1) * 128],
                                      kp[:])
            else:
                nc.scalar.copy(ckTn_ext[0:DA, s * 128:(s + 1) * 128], kp[:])

        def emit_dot(s):
            b0, b1 = _bfirst(s), _blast(s)
            gcnt = b1 - b0 + 1
            dps = dotps.tile([128, 32 * gcnt], F32, space="PSUM",
                             tag="dot", bufs=2)
            nc.tensor.matmul(dps[:], ckTn_ext[:, s * 128:(s + 1) * 128],
                             tqnT_ext[:, b0 * J:(b1 + 1) * J],
                             start=True, stop=True)
            et = etp.tile([128, 32 * gcnt], BF16, tag="et", bufs=6)
            et_tiles[s] = et
            nc.scalar.activation(et[:], dps[:], AF.Exp, scale=1.0)

        def emit_z(s):
            b0, b1 = _bfirst(s), _blast(s)
            et = et_tiles.pop(s)
            for g in range(b1 - b0 + 1):
                b = b0 + g
                q = b % 3
                grp = b // 3
                if grp not in zp3_tiles:
                    zp3_tiles[grp] = zps_p.tile([96, E + 1], F32,
                                                space="PSUM", tag="z", bufs=3,
                                                name=f"zp3_{grp}")
                zp3 = zp3_tiles[grp]
                s0 = (b * M) // 128
                s1 = (b * M + M - 1) // 128
                nc.tensor.matmul(zp3[32 * q:32 * (q + 1), :],
                                 et[:, 32 * g:32 * (g + 1)],
                                 craw_tiles[s][:, 0:E + 1],
                                 start=(s == s0), stop=(s == s1))

        def emit_fin(grp):
            zp3 = zp3_tiles.pop(grp)
            nb = min(3 * grp + 3, BL) - 3 * grp     # 3, or 2 in last group
            r = 32 * nb
            inv = workp.tile([96, 1], F32, tag="inv", bufs=4)
            nc.vector.reciprocal(inv[:r], zp3[:r, E:E + 1])
            zsb = workp.tile([96, E], F32, tag="zsb", bufs=4)
            nc.vector.tensor_scalar_mul(zsb[:r], zp3[:r, 0:E], inv[:r, :1])
            nc.vector.tensor_tensor(out=zsb[:r], in0=zsb[:r],
                                    in1=b2_t[:r], op=OP.add)
            nc.sync.dma_start(out=zout[3 * grp:3 * grp + nb], in_=zsb[:r])

        # -------- software-pipelined schedule (2-tile stage lag) --------
        next_t = 0
        next_fin = 0
        for i in range(NT_C + 2):
            if i < NT_C:
                emit_c(i)
                if i % 3 == 1 and next_t < NT_T:
                    emit_t(next_t)
                    next_t += 1
                    if i == 1:          # front-load a second t tile
                        emit_t(next_t)
                        next_t += 1
            if 1 <= i <= NT_C:
                emit_dot(i - 1)
            if i >= 2:
                emit_z(i - 2)
                while next_fin < NGRP and \
                        (min(3 * next_fin + 2, BL - 1) * M + M - 1) // 128 \
                        <= i - 2:
                    emit_fin(next_fin)
                    next_fin += 1
        while next_t < NT_T:
            emit_t(next_t)
            next_t += 1
        while next_fin < NGRP:
            emit_fin(next_fin)
            next_fin += 1

        ctx.close()

    nc.finalize()
    return nc


_nc_cache = [None]


def kernel(batch_titems, batch_citems, pad_rows, pad_cols, tvec, cvec,
           Ac_w, Ac_b, At_w, At_b, Bc_w, Bc_b, R_w, R_b):
    batch_titems = np.asarray(batch_titems).astype(np.int32)
    batch_citems = np.asarray(batch_citems).astype(np.int32)
    pad_rows = np.asarray(pad_rows).astype(np.int64)
    pad_cols = np.asarray(pad_cols).astype(np.int64)
    tvec = np.asarray(tvec, dtype=np.float32)
    cvec = np.asarray(cvec, dtype=np.float32)
    Ac_w = np.asarray(Ac_w, dtype=np.float32)
    Ac_b = np.asarray(Ac_b, dtype=np.float32)
    At_w = np.asarray(At_w, dtype=np.float32)
    At_b = np.asarray(At_b, dtype=np.float32)
    Bc_w = np.asarray(Bc_w, dtype=np.float32)
    Bc_b = np.asarray(Bc_b, dtype=np.float32)
    R_w = np.asarray(R_w, dtype=np.float32)
    R_b = np.asarray(R_b, dtype=np.float32)

    # ---- host table folding ----
    W2 = R_w @ Bc_w                                   # [E, E]
    b2 = (R_w @ Bc_b + R_b).astype(np.float32)        # [E]
    bu2 = (cvec @ W2.T).astype(np.float32)            # [V, E]
    ck = cvec @ Ac_w.T + Ac_b                         # [V, DA]
    ck /= np.maximum(np.linalg.norm(ck, axis=1, keepdims=True), EPS)
    tq = tvec @ At_w.T + At_b                         # [V, DA]
    tq /= np.maximum(np.linalg.norm(tq, axis=1, keepdims=True), EPS)
    ttab = tq.astype(ml_dtypes.bfloat16)

    ctab = np.zeros((V, CW), dtype=ml_dtypes.bfloat16)
    ctab[:, 0:E] = bu2.astype(ml_dtypes.bfloat16)
    ctab[:, E] = np.asarray(1.0, dtype=ml_dtypes.bfloat16)
    ctab[:, 132:CW] = ck.astype(ml_dtypes.bfloat16)

    b2rep = np.broadcast_to(b2, (96, E)).copy()
    ident_np = np.eye(128, dtype=np.float32).astype(ml_dtypes.bfloat16)

    # block-ones rank-3 factor: [r, (b, j)] = 1 iff b % 3 == r
    bones = np.zeros((3, BL * J), dtype=np.float32)
    bb = np.repeat(np.arange(BL), J)
    bones[bb % 3, np.arange(BL * J)] = 1.0
    bones = bones.astype(ml_dtypes.bfloat16)

    in_maps = []
    toks = np.arange(BL * M)
    tok_b_own = toks // M
    for c in range(NCORES):
        b0c = c * BL
        cit = batch_citems[b0c:b0c + BL].ravel()      # [12800]
        tit = batch_titems[b0c:b0c + BL].ravel()      # [4096]
        cidx = np.ascontiguousarray(cit.reshape(NT_C, 128).T.astype(np.int32))
        tidx = np.ascontiguousarray(tit.reshape(NT_T, 128).T.astype(np.int32))
        sel = (pad_rows >= b0c) & (pad_rows < b0c + BL)
        negm = np.zeros((M, BL), dtype=np.float32)
        negm[pad_cols[sel], pad_rows[sel] - b0c] = NEG
        # rank-3 bias factor rows: negs3[r, tok] = bias of token tok w.r.t.
        # the unique b in its tile's group with b % 3 == r (NEG otherwise)
        negs3 = np.full((3, BL * M), NEG, dtype=np.float32)
        for s in range(NT_C):
            tsl = slice(128 * s, 128 * (s + 1))
            t = toks[tsl]
            for bg in range(_bfirst(s), _blast(s) + 1):
                own = tok_b_own[tsl] == bg
                col = np.full(128, NEG, dtype=np.float32)
                col[own] = negm[t[own] - M * bg, bg]
                negs3[bg % 3, tsl] = col
        in_maps.append({
            "ctab": ctab, "ttab": ttab,
            "cidx": cidx, "tidx": tidx,
            "negs3d": negs3.astype(ml_dtypes.bfloat16), "bonesd": bones,
            "b2d": b2rep, "identd": ident_np,
        })

    if _nc_cache[0] is None:
        _nc_cache[0] = _build_bass()
    nc = _nc_cache[0]

    res = run_bass_kernel_spmd(nc, in_maps, list(range(NCORES)),
                               trace=_trace[0])
    _last_exec_ns[0] = res.exec_time_ns
    z = np.concatenate([r["zout"] for r in res.results], axis=0)
    return z.astype(np.float32)
